# revision 1
# baseline (speedup 1.0000x reference)
"""Trainium2 Bass kernel v2 for nn_Decoder_70781061038698.

2-layer peephole LSTM decoder, 63 sequential steps.
Strategy: hybrid shard over 8 cores = 4 batch groups x 2 unit-halves.
Each pair of cores (2p, 2p+1) handles batch group p (128 rows); within the
pair, core u owns units [512u : 512u+512) of both layers. ALL weights stay
resident in SBUF (8.4 MB/core) - zero steady-state weight DMA. Batch M=128
fills the full PE array. Per step, each layer's (c_new, h_new) transposed
halves are AllGather'd within the pair (2 collectives/step).

Self-contained: hardcodes all shapes; no imports from /root/problem.
"""

import numpy as np
import ml_dtypes

import concourse.bass as bass
import concourse.mybir as mybir
import concourse.tile as tile
from concourse import bacc
from concourse.masks import make_identity

N_CORES = 8
BS = 512
BG = 128             # batch per pair (= per core, unit-sharded)
UNITS = 1024
UH = 512             # units per core
VDIM = 974
BDIM = 50
FORGET_BIAS = 0.8
GATES = 4
GATE_ORDER = [1, 0, 2, 3]   # j, i, f, o

F32 = mybir.dt.float32
BF16 = mybir.dt.bfloat16
Tanh = mybir.ActivationFunctionType.Tanh
Sigmoid = mybir.ActivationFunctionType.Sigmoid

REPLICA_GROUPS = [[0, 1], [2, 3], [4, 5], [6, 7]]


def _build(T: int):
    nc = bacc.Bacc("TRN2", target_bir_lowering=False, debug=False,
                   num_devices=N_CORES)

    def din(name, shape, dt):
        return nc.dram_tensor(name, list(shape), dt, kind="ExternalInput").ap()

    vtp = din("vtp", (T, 128, 1024), BF16)    # v^T chunks per step
    w1r = din("w1r", (GATES, 128, 16 * 512), BF16)
    w1o = din("w1o", (64, GATES * 512), BF16)  # out rows (974:1024 padded)
    b1a = din("b1a", (1, GATES * 512), BF16)
    w2r = din("w2r", (GATES, 128, 16 * 512), BF16)
    b2a = din("b2a", (1, GATES * 512), BF16)
    wlT = din("wlT", (128, 8 * 64), BF16)
    blT = din("blT", (1, 64), BF16)
    c10 = din("c10", (128, UH), F32)
    c20 = din("c20", (128, UH), F32)
    h10T = din("h10T", (128, 1024), BF16)      # full h-slot^T, layer 1
    h20T = din("h20T", (128, 1024), BF16)
    o0T = din("o0T", (64, 128), BF16)
    pp = {}
    for l in (1, 2):
        for nm in ("pi", "pf", "po"):
            pp[(nm, l)] = din(f"{nm}{l}", (128, UH), BF16)

    ysT = nc.dram_tensor("ysT", [T, 64, 128], F32, kind="ExternalOutput").ap()

    with tile.TileContext(nc) as tc:
        with (
            tc.tile_pool(name="const", bufs=1) as constp,
            tc.tile_pool(name="wres", bufs=1) as wrp,
            tc.tile_pool(name="xv", bufs=2) as xvp,
            tc.tile_pool(name="hT", bufs=2) as hTp,
            tc.tile_pool(name="myT", bufs=2) as myTp,
            tc.tile_pool(name="st", bufs=2) as stp,
            tc.tile_pool(name="cn", bufs=2) as cnp,
            tc.tile_pool(name="tmp", bufs=5) as tmpp,
            tc.tile_pool(name="outp", bufs=2) as outp,
            tc.tile_pool(name="gpsum", bufs=5, space="PSUM") as gpsp,
            tc.tile_pool(name="trpsum", bufs=2, space="PSUM") as trpsp,
            tc.tile_pool(name="opsum", bufs=1, space="PSUM") as opsp,
            tc.tile_pool(name="dram", bufs=2, space="DRAM") as dramp,
        ):
            # ---- constants ----
            ident = constp.tile([128, 128], F32, tag="ident", name="ident")
            make_identity(nc, ident[:])
            ones = constp.tile([1, 128], BF16, tag="ones", name="ones")
            nc.gpsimd.memset(ones[:], 1.0)

            wt = {}
            for l, src in ((1, w1r), (2, w2r)):
                for g in range(GATES):
                    t_ = wrp.tile([128, 16 * 512], BF16, tag=f"w{l}{g}",
                                  name=f"w{l}{g}")
                    nc.sync.dma_start(t_[:], src[g])
                    wt[(l, g)] = t_
            w1ot = constp.tile([64, GATES * 512], BF16, tag="w1o", name="w1o")
            nc.sync.dma_start(w1ot[:], w1o[:])
            bat = {}
            for l, src in ((1, b1a), (2, b2a)):
                t_ = constp.tile([1, GATES * 512], BF16, tag=f"ba{l}", name=f"ba{l}")
                nc.sync.dma_start(t_[:], src[:])
                bat[l] = t_
            wlt = constp.tile([128, 8 * 64], BF16, tag="wl", name="wl")
            nc.sync.dma_start(wlt[:], wlT[:])
            blt = constp.tile([1, 64], BF16, tag="bl", name="bl")
            nc.sync.dma_start(blt[:], blT[:])
            ppt = {}
            for key, ap in pp.items():
                t_ = constp.tile([128, UH], BF16, tag=f"pp{key[0]}{key[1]}",
                                 name=f"pp{key[0]}{key[1]}")
                nc.sync.dma_start(t_[:], ap[:])
                ppt[key] = t_

            # ---- initial state ----
            c_sl = {}     # local c-slot input (h_new of prev step), [128, UH] f32
            for l, src in ((1, c10), (2, c20)):
                t_ = stp.tile([128, UH], F32, tag=f"c{l}", name=f"c{l}")
                nc.sync.dma_start(t_[:], src[:])
                c_sl[l] = t_
            hslotT = {}   # full h-slot^T (c_new of prev step), [128, 1024] bf16
            for l, src in ((1, h10T), (2, h20T)):
                t_ = hTp.tile([128, 1024], BF16, tag=f"h{l}T", name=f"h{l}T")
                nc.sync.dma_start(t_[:], src[:])
                hslotT[l] = t_
            out_t = outp.tile([64, 128], BF16, tag="outT", name="outT")
            nc.sync.dma_start(out_t[:], o0T[:])

            def cell_elementwise(l, ps, c_cur):
                """Peephole cell on [128, UH] tiles. ps: gate->psum.
                Returns (c_new, h_new)."""
                def tmp():
                    return tmpp.tile([128, UH], F32, tag="tmp", name="tmp")
                tj = tmp()
                nc.scalar.activation(tj[:], ps[1][:], Tanh)
                tf = tmp()
                nc.gpsimd.tensor_mul(tf[:], c_cur[:], ppt[("pf", l)][:])
                fa = tmp()
                nc.vector.tensor_add(fa[:], tf[:], ps[2][:])
                fs = tmp()
                nc.scalar.activation(fs[:], fa[:], Sigmoid)
                ti = tmp()
                nc.gpsimd.tensor_mul(ti[:], c_cur[:], ppt[("pi", l)][:])
                ia = tmp()
                nc.vector.tensor_add(ia[:], ti[:], ps[0][:])
                is_ = tmp()
                nc.scalar.activation(is_[:], ia[:], Sigmoid)
                t1 = tmp()
                nc.vector.tensor_mul(t1[:], fs[:], c_cur[:])
                t2 = tmp()
                nc.vector.tensor_mul(t2[:], is_[:], tj[:])
                c_new = cnp.tile([128, UH], F32, tag=f"cn{l}", name=f"cn{l}")
                nc.vector.tensor_add(c_new[:], t1[:], t2[:])
                to = tmp()
                nc.gpsimd.tensor_mul(to[:], c_new[:], ppt[("po", l)][:])
                oa = tmp()
                nc.vector.tensor_add(oa[:], to[:], ps[3][:])
                os_ = tmp()
                nc.scalar.activation(os_[:], oa[:], Sigmoid)
                ct = tmp()
                nc.scalar.activation(ct[:], c_new[:], Tanh)
                h_new = stp.tile([128, UH], F32, tag=f"c{l}", name=f"c{l}")
                nc.vector.tensor_mul(h_new[:], os_[:], ct[:])
                return c_new, h_new

            def exchange(l, c_new, h_new):
                """Transpose c_new/h_new halves, AllGather within pair.
                Returns (cT_full, hT_full) tiles [128, 1024] bf16 each,
                packed as one [128, 2048] tile: [cT | hT]."""
                trc = trpsp.tile([128, 512], F32, tag="tr", name="tr")
                trh = trpsp.tile([128, 512], F32, tag="tr", name="tr")
                for b in range(4):
                    nc.tensor.transpose(trc[:, 128 * b:128 * b + 128],
                                        c_new[:, 128 * b:128 * b + 128],
                                        ident[:])
                for b in range(4):
                    nc.tensor.transpose(trh[:, 128 * b:128 * b + 128],
                                        h_new[:, 128 * b:128 * b + 128],
                                        ident[:])
                myt = myTp.tile([128, 1024], BF16, tag="myT", name="myT")
                nc.vector.tensor_copy(myt[:, 0:512], trc[:])
                nc.scalar.copy(myt[:, 512:1024], trh[:])
                cin = dramp.tile([128, 1024], BF16, tag="ci", name="ci")
                cout = dramp.tile([2, 128, 1024], BF16, tag="co", name="co")
                nc.sync.dma_start(cin[:], myt[:])
                nc.gpsimd.collective_compute(
                    "AllGather", mybir.AluOpType.bypass,
                    replica_groups=REPLICA_GROUPS,
                    ins=[cin[:].opt()], outs=[cout[:].opt()],
                )
                gath = hTp.tile([128, 2048], BF16, tag=f"g{l}", name=f"g{l}")
                # dst[p, x*1024 + r*512 + n] = cout[r, p, x*512 + n]
                nc.sync.dma_start(
                    gath[:].rearrange("p (x r n) -> p x r n", x=2, r=2),
                    cout.rearrange("r p (x n) -> p x r n", x=2),
                )
                return gath[:, 0:1024], gath[:, 1024:2048]

            # ------------------------------------------------------------------
            x2T = None   # full h_new(L1)^T for W2 rows 0:1024
            for t in range(T):
                xv = xvp.tile([128, 1024], BF16, tag="xv", name="xv")
                nc.sync.dma_start(xv[:], vtp[t])

                # ---- Layer 1 (chunk-major: one stationary, 4 gate MMs) ----
                ps = {g: gpsp.tile([128, 512], F32, tag="g", name="g")
                      for g in GATE_ORDER}
                for k in range(8):         # v chunks
                    for g in GATE_ORDER:
                        nc.tensor.matmul(ps[g][:], xv[:, 128 * k:128 * k + 128],
                                         wt[(1, g)][:, 512 * k:512 * k + 512],
                                         start=(k == 0), stop=False,
                                         skip_group_check=True)
                for k in range(8):         # h-slot chunks (c1_new prev)
                    for g in GATE_ORDER:
                        nc.tensor.matmul(ps[g][:],
                                         hslotT[1][:, 128 * k:128 * k + 128],
                                         wt[(1, g)][:, 512 * (8 + k):512 * (8 + k) + 512],
                                         start=False, stop=False,
                                         skip_group_check=True)
                for g in GATE_ORDER:
                    nc.tensor.matmul(ps[g][:], ones[:],
                                     bat[1][:, 512 * g:512 * g + 512],
                                     start=False, stop=False,
                                     skip_group_check=True)
                for g in GATE_ORDER:
                    nc.tensor.matmul(ps[g][:], out_t[:],
                                     w1ot[:, 512 * g:512 * g + 512],
                                     start=False, stop=True,
                                     skip_group_check=True)
                ps1 = ps

                # ---- Layer 2 independent part (h-slot chunks + bias) ----
                # Emitted before L1's elementwise-dependent transposes so the
                # in-order PE queue runs these while other engines do L1's
                # cell math.
                ps = {g: gpsp.tile([128, 512], F32, tag="g", name="g")
                      for g in GATE_ORDER}
                for k in range(8):         # h-slot chunks (c2_new prev) FIRST
                    for g in GATE_ORDER:
                        nc.tensor.matmul(ps[g][:],
                                         hslotT[2][:, 128 * k:128 * k + 128],
                                         wt[(2, g)][:, 512 * k:512 * k + 512],
                                         start=(k == 0), stop=False,
                                         skip_group_check=True)
                for g in GATE_ORDER:
                    nc.tensor.matmul(ps[g][:], ones[:],
                                     bat[2][:, 512 * g:512 * g + 512],
                                     start=False, stop=False,
                                     skip_group_check=True)

                c1n, h1n = cell_elementwise(1, ps1, c_sl[1])
                c_sl[1] = h1n
                c1T, x2T = exchange(1, c1n, h1n)
                hslotT[1] = c1T

                # ---- Layer 2 dependent part (h1/x chunks) ----
                for k in range(8):         # h1 (x) chunks, wait on L1 exchange
                    for g in GATE_ORDER:
                        nc.tensor.matmul(ps[g][:], x2T[:, 128 * k:128 * k + 128],
                                         wt[(2, g)][:, 512 * (8 + k):512 * (8 + k) + 512],
                                         start=False, stop=(k == 7),
                                         skip_group_check=True)
                c2n, h2n = cell_elementwise(2, ps, c_sl[2])
                c_sl[2] = h2n
                c2T, h2T = exchange(2, c2n, h2n)
                hslotT[2] = c2T

                # ---- output projection: out^T = tanh(Wl^T @ h2 + bl) ----
                pso = opsp.tile([64, 128], F32, tag="op", name="op")
                for k in range(8):
                    nc.tensor.matmul(pso[:], wlt[:, 64 * k:64 * k + 64],
                                     h2T[:, 128 * k:128 * k + 128],
                                     start=(k == 0), stop=False)
                nc.tensor.matmul(pso[:], blt[:], ones[:],
                                 start=False, stop=True)
                ys_sb = outp.tile([64, 128], F32, tag="ysb", name="ysb")
                nc.scalar.activation(ys_sb[:], pso[:], Tanh)
                nc.sync.dma_start(ysT[t], ys_sb[:])
                if t + 1 < T:
                    out_t = outp.tile([64, 128], BF16, tag="outT", name="outT")
                    nc.vector.tensor_copy(out_t[:], ys_sb[:])

    nc.compile()
    return nc


# ---------------------------------------------------------------------------
# host-side input prep
# ---------------------------------------------------------------------------

def _bf16(x):
    return np.asarray(x, dtype=np.float32).astype(ml_dtypes.bfloat16)


def _prep_inputs(inputs, T):
    V = np.asarray(inputs["V_seq"], dtype=np.float32)
    B0 = np.asarray(inputs["B0"], dtype=np.float32)
    init = np.asarray(inputs["initial_state"], dtype=np.float32)
    init = init.reshape(BS, 2, 2, UNITS)
    W1 = np.asarray(inputs["W1"], dtype=np.float32)
    W2 = np.asarray(inputs["W2"], dtype=np.float32)
    b1 = np.asarray(inputs["b1"], dtype=np.float32)
    b2 = np.asarray(inputs["b2"], dtype=np.float32)
    Wl = np.asarray(inputs["Wl"], dtype=np.float32)
    bl = np.asarray(inputs["bl"], dtype=np.float32)

    W1z = W1.copy()
    W1z[VDIM:1024] = 0.0
    arr1 = W1z.reshape(16, 128, 4096)
    arr2 = W2.reshape(16, 128, 4096)

    fb = np.concatenate([np.zeros(2048, np.float32),
                         np.full(1024, FORGET_BIAS, np.float32),
                         np.zeros(1024, np.float32)])
    b1f = b1 + fb
    b2f = b2 + fb

    # Wl^T tile: [p, 64k + o] = Wl[128k + p, o]
    wlT = np.zeros((128, 8, 64), np.float32)
    wlT[:, :, :BDIM] = Wl.reshape(8, 128, BDIM).transpose(1, 0, 2)
    wlT = _bf16(wlT.reshape(128, 512))
    blT = np.zeros((1, 64), np.float32)
    blT[0, :BDIM] = bl
    blT = _bf16(blT)

    in_maps = []
    for c in range(N_CORES):
        p, u = c // 2, c % 2
        b0 = p * BG
        cols = np.concatenate([np.arange(g * 1024 + u * UH, g * 1024 + u * UH + UH)
                               for g in range(GATES)])  # (2048,)

        Vp = np.zeros((T, BG, 1024), np.float32)
        Vp[:, :, :VDIM] = V[:T, b0:b0 + BG, :]
        vtp = Vp.reshape(T, BG, 8, 128).transpose(0, 3, 2, 1).reshape(T, 128, 1024)

        w1c = arr1[:, :, cols].reshape(16, 128, 4, 512)    # [k, p, g, n]
        w1rc = w1c.transpose(2, 1, 0, 3).reshape(4, 128, 8192)
        # L2: chunk order [h-slot rows 1024:2048 (k 8..15) | h1 rows 0:1024]
        w2c = arr2[:, :, cols].reshape(16, 128, 4, 512)
        w2rc = np.concatenate([w2c[8:16], w2c[0:8]], axis=0)
        w2rc = w2rc.transpose(2, 1, 0, 3).reshape(4, 128, 8192)

        w1oc = np.zeros((64, 4, 512), np.float32)
        w1oc[:BDIM] = W1[VDIM:VDIM + BDIM][:, cols].reshape(BDIM, 4, 512)
        w1oc = w1oc.reshape(64, 2048)

        h0T = {}
        for l in (1, 2):
            hs = init[b0:b0 + BG, l - 1, 1, :]          # (128, 1024)
            h0T[l] = hs.T.reshape(8, 128, BG).transpose(1, 0, 2).reshape(128, 1024)

        o0T = np.zeros((64, 128), np.float32)
        o0T[:BDIM] = B0[b0:b0 + BG].T

        m = {
            "vtp": _bf16(vtp),
            "w1r": _bf16(w1rc), "w1o": _bf16(w1oc), "b1a": _bf16(b1f[cols][None, :]),
            "w2r": _bf16(w2rc), "b2a": _bf16(b2f[cols][None, :]),
            "wlT": wlT, "blT": blT,
            "c10": np.ascontiguousarray(init[b0:b0 + BG, 0, 0, u * UH:u * UH + UH]),
            "c20": np.ascontiguousarray(init[b0:b0 + BG, 1, 0, u * UH:u * UH + UH]),
            "h10T": _bf16(h0T[1]), "h20T": _bf16(h0T[2]),
            "o0T": _bf16(o0T),
        }
        for l in (1, 2):
            for nm in ("pi", "pf", "po"):
                v = np.asarray(inputs[f"{nm}{l}"], dtype=np.float32)
                m[f"{nm}{l}"] = _bf16(np.broadcast_to(v[u * UH:u * UH + UH],
                                                      (128, UH)))
        in_maps.append(m)
    return in_maps


# ---------------------------------------------------------------------------
# cached PJRT runner
# ---------------------------------------------------------------------------

_RUNNERS = {}


class _Runner:
    def __init__(self, T):
        import jax
        from jax.sharding import Mesh, PartitionSpec, NamedSharding
        from jax.experimental.shard_map import shard_map
        from concourse import bass2jax

        self.T = T
        nc = _build(T)
        bass2jax.install_neuronx_cc_hook()

        partition_name = (nc.partition_id_tensor.name
                          if nc.partition_id_tensor else None)
        in_names, out_names, out_avals, zero_outs = [], [], [], []
        for alloc in nc.m.functions[0].allocations:
            if not isinstance(alloc, mybir.MemoryLocationSet):
                continue
            name = alloc.memorylocations[0].name
            if alloc.kind == "ExternalInput":
                if name != partition_name:
                    in_names.append(name)
            elif alloc.kind == "ExternalOutput":
                shape = tuple(alloc.tensor_shape)
                dtype = mybir.dt.np(alloc.dtype)
                out_names.append(name)
                out_avals.append(jax.core.ShapedArray(shape, dtype))
                zero_outs.append(np.zeros(shape, dtype))
        self.in_names = list(in_names)
        self.out_names = out_names
        self.zero_outs = zero_outs
        n_params = len(in_names)
        n_outs = len(out_avals)
        all_in_names = list(in_names) + list(out_names)
        if partition_name is not None:
            all_in_names.append(partition_name)
        donate = tuple(range(n_params, n_params + n_outs))

        def _body(*args):
            operands = list(args)
            if partition_name is not None:
                operands.append(bass2jax.partition_id_tensor())
            return tuple(bass2jax._bass_exec_p.bind(
                *operands,
                out_avals=tuple(out_avals),
                in_names=tuple(all_in_names),
                out_names=tuple(out_names),
                lowering_input_output_aliases=(),
                sim_require_finite=True,
                sim_require_nnan=True,
                nc=nc,
            ))

        devices = jax.devices()[:N_CORES]
        assert len(devices) == N_CORES
        self.mesh = Mesh(np.asarray(devices), ("core",))
        in_specs = (PartitionSpec("core"),) * (n_params + n_outs)
        out_specs = (PartitionSpec("core"),) * n_outs
        self.sharding = NamedSharding(self.mesh, PartitionSpec("core"))
        self.fn = jax.jit(
            shard_map(_body, mesh=self.mesh, in_specs=in_specs,
                      out_specs=out_specs, check_rep=False),
            donate_argnums=donate, keep_unused=True)
        self.jax = jax

    def device_inputs(self, in_maps):
        cat = [np.concatenate([np.asarray(m[n]) for m in in_maps], axis=0)
               for n in self.in_names]
        return [self.jax.device_put(a, self.sharding) for a in cat]

    def zero_out_bufs(self):
        cat = [np.concatenate([z] * N_CORES, axis=0) for z in self.zero_outs]
        return [self.jax.device_put(a, self.sharding) for a in cat]

    def run(self, dev_in, out_bufs):
        outs = self.fn(*dev_in, *out_bufs)
        return [np.asarray(o) for o in outs]


def _get_runner(T):
    if T not in _RUNNERS:
        _RUNNERS[T] = _Runner(T)
    return _RUNNERS[T]


# ---------------------------------------------------------------------------
# public entry point
# ---------------------------------------------------------------------------

def kernel(**inputs) -> np.ndarray:
    T = int(inputs["length"]) - 1
    if T <= 0:
        return np.zeros((max(T, 0), BS, BDIM), dtype=np.float32)
    runner = _get_runner(T)
    in_maps = _prep_inputs(inputs, T)
    dev_in = runner.device_inputs(in_maps)
    outs = runner.run(dev_in, runner.zero_out_bufs())
    ysT_cat = outs[runner.out_names.index("ysT")]   # (8*T, 64, 128)
    per_core = ysT_cat.reshape(N_CORES, T, 64, 128)
    ys = np.empty((T, BS, BDIM), dtype=np.float32)
    for p in range(4):
        ys[:, p * BG:(p + 1) * BG, :] = (
            per_core[2 * p, :, :BDIM, :].transpose(0, 2, 1))
    return ys



# revision 2
# speedup vs baseline: 16.9999x; 16.9999x over previous
"""Trainium2 Bass kernel v2 for nn_Decoder_70781061038698.

2-layer peephole LSTM decoder, 63 sequential steps.
Strategy: hybrid shard over 8 cores = 4 batch groups x 2 unit-halves.
Each pair of cores (2p, 2p+1) handles batch group p (128 rows); within the
pair, core u owns units [512u : 512u+512) of both layers. ALL weights stay
resident in SBUF (8.4 MB/core) - zero steady-state weight DMA. Batch M=128
fills the full PE array. Per step, each layer's (c_new, h_new) transposed
halves are AllGather'd within the pair (2 collectives/step).

Self-contained: hardcodes all shapes; no imports from /root/problem.
"""

import numpy as np
import ml_dtypes

import concourse.bass as bass
import concourse.mybir as mybir
import concourse.tile as tile
from concourse import bacc
from concourse.masks import make_identity

N_CORES = 8
BS = 512
BG = 128             # batch per pair (= per core, unit-sharded)
UNITS = 1024
UH = 512             # units per core
VDIM = 974
BDIM = 50
FORGET_BIAS = 0.8
GATES = 4
GATE_ORDER = [1, 0, 2, 3]   # j, i, f, o

F32 = mybir.dt.float32
BF16 = mybir.dt.bfloat16
Tanh = mybir.ActivationFunctionType.Tanh
Sigmoid = mybir.ActivationFunctionType.Sigmoid

REPLICA_GROUPS = [[0, 1], [2, 3], [4, 5], [6, 7]]


def _build(T: int):
    nc = bacc.Bacc("TRN2", target_bir_lowering=False, debug=False,
                   num_devices=N_CORES)

    def din(name, shape, dt):
        return nc.dram_tensor(name, list(shape), dt, kind="ExternalInput").ap()

    vtp = din("vtp", (T, 128, 1024), BF16)    # v^T chunks per step
    w1r = din("w1r", (GATES, 128, 16 * 512), BF16)
    w1o = din("w1o", (64, GATES * 512), BF16)  # out rows (974:1024 padded)
    b1a = din("b1a", (1, GATES * 512), BF16)
    w2r = din("w2r", (GATES, 128, 16 * 512), BF16)
    b2a = din("b2a", (1, GATES * 512), BF16)
    wlT = din("wlT", (128, 8 * 64), BF16)
    blT = din("blT", (1, 64), BF16)
    c10 = din("c10", (128, UH), F32)
    c20 = din("c20", (128, UH), F32)
    h10T = din("h10T", (128, 1024), BF16)      # full h-slot^T, layer 1
    h20T = din("h20T", (128, 1024), BF16)
    o0T = din("o0T", (64, 128), BF16)
    pp = {}
    for l in (1, 2):
        for nm in ("pi", "pf", "po"):
            pp[(nm, l)] = din(f"{nm}{l}", (128, UH), BF16)

    ysT = nc.dram_tensor("ysT", [T, 64, 128], F32, kind="ExternalOutput").ap()

    with tile.TileContext(nc) as tc:
        with (
            tc.tile_pool(name="const", bufs=1) as constp,
            tc.tile_pool(name="wres", bufs=1) as wrp,
            tc.tile_pool(name="xv", bufs=2) as xvp,
            tc.tile_pool(name="hT", bufs=2) as hTp,
            tc.tile_pool(name="myT", bufs=2) as myTp,
            tc.tile_pool(name="st", bufs=2) as stp,
            tc.tile_pool(name="cn", bufs=2) as cnp,
            tc.tile_pool(name="tmp", bufs=5) as tmpp,
            tc.tile_pool(name="outp", bufs=2) as outp,
            tc.tile_pool(name="gpsum", bufs=5, space="PSUM") as gpsp,
            tc.tile_pool(name="trpsum", bufs=2, space="PSUM") as trpsp,
            tc.tile_pool(name="opsum", bufs=1, space="PSUM") as opsp,
            tc.tile_pool(name="dram", bufs=2, space="DRAM") as dramp,
        ):
            # ---- constants ----
            ident = constp.tile([128, 128], F32, tag="ident", name="ident")
            make_identity(nc, ident[:])
            ones = constp.tile([1, 128], BF16, tag="ones", name="ones")
            nc.gpsimd.memset(ones[:], 1.0)

            wt = {}
            for l, src in ((1, w1r), (2, w2r)):
                for g in range(GATES):
                    t_ = wrp.tile([128, 16 * 512], BF16, tag=f"w{l}{g}",
                                  name=f"w{l}{g}")
                    nc.sync.dma_start(t_[:], src[g])
                    wt[(l, g)] = t_
            w1ot = constp.tile([64, GATES * 512], BF16, tag="w1o", name="w1o")
            nc.sync.dma_start(w1ot[:], w1o[:])
            bat = {}
            for l, src in ((1, b1a), (2, b2a)):
                t_ = constp.tile([1, GATES * 512], BF16, tag=f"ba{l}", name=f"ba{l}")
                nc.sync.dma_start(t_[:], src[:])
                bat[l] = t_
            wlt = constp.tile([128, 8 * 64], BF16, tag="wl", name="wl")
            nc.sync.dma_start(wlt[:], wlT[:])
            blt = constp.tile([1, 64], BF16, tag="bl", name="bl")
            nc.sync.dma_start(blt[:], blT[:])
            ppt = {}
            for key, ap in pp.items():
                t_ = constp.tile([128, UH], BF16, tag=f"pp{key[0]}{key[1]}",
                                 name=f"pp{key[0]}{key[1]}")
                nc.sync.dma_start(t_[:], ap[:])
                ppt[key] = t_

            # ---- initial state ----
            c_sl = {}     # local c-slot input (h_new of prev step), [128, UH] f32
            for l, src in ((1, c10), (2, c20)):
                t_ = stp.tile([128, UH], F32, tag=f"c{l}", name=f"c{l}")
                nc.sync.dma_start(t_[:], src[:])
                c_sl[l] = t_
            hslotT = {}   # full h-slot^T (c_new of prev step), [128, 1024] bf16
            for l, src in ((1, h10T), (2, h20T)):
                t_ = hTp.tile([128, 1024], BF16, tag=f"h{l}T", name=f"h{l}T")
                nc.sync.dma_start(t_[:], src[:])
                hslotT[l] = t_
            out_t = outp.tile([64, 128], BF16, tag="outT", name="outT")
            nc.sync.dma_start(out_t[:], o0T[:])

            def cell_elementwise(l, ps, c_cur):
                """Peephole cell on [128, UH] tiles. ps: gate->psum.
                Returns (c_new, h_new)."""
                def tmp():
                    return tmpp.tile([128, UH], F32, tag="tmp", name="tmp")
                tj = tmp()
                nc.scalar.activation(tj[:], ps[1][:], Tanh)
                tf = tmp()
                nc.gpsimd.tensor_mul(tf[:], c_cur[:], ppt[("pf", l)][:])
                fa = tmp()
                nc.vector.tensor_add(fa[:], tf[:], ps[2][:])
                fs = tmp()
                nc.scalar.activation(fs[:], fa[:], Sigmoid)
                ti = tmp()
                nc.gpsimd.tensor_mul(ti[:], c_cur[:], ppt[("pi", l)][:])
                ia = tmp()
                nc.vector.tensor_add(ia[:], ti[:], ps[0][:])
                is_ = tmp()
                nc.scalar.activation(is_[:], ia[:], Sigmoid)
                t1 = tmp()
                nc.vector.tensor_mul(t1[:], fs[:], c_cur[:])
                t2 = tmp()
                nc.vector.tensor_mul(t2[:], is_[:], tj[:])
                c_new = cnp.tile([128, UH], F32, tag=f"cn{l}", name=f"cn{l}")
                nc.vector.tensor_add(c_new[:], t1[:], t2[:])
                to = tmp()
                nc.gpsimd.tensor_mul(to[:], c_new[:], ppt[("po", l)][:])
                oa = tmp()
                nc.vector.tensor_add(oa[:], to[:], ps[3][:])
                os_ = tmp()
                nc.scalar.activation(os_[:], oa[:], Sigmoid)
                ct = tmp()
                nc.scalar.activation(ct[:], c_new[:], Tanh)
                h_new = stp.tile([128, UH], F32, tag=f"c{l}", name=f"c{l}")
                nc.vector.tensor_mul(h_new[:], os_[:], ct[:])
                return c_new, h_new

            def exchange(l, c_new, h_new):
                """Transpose c_new/h_new halves, AllGather within pair.
                Returns (cT_full, hT_full) tiles [128, 1024] bf16 each,
                packed as one [128, 2048] tile: [cT | hT]."""
                trc = trpsp.tile([128, 512], F32, tag="tr", name="tr")
                trh = trpsp.tile([128, 512], F32, tag="tr", name="tr")
                for b in range(4):
                    nc.tensor.transpose(trc[:, 128 * b:128 * b + 128],
                                        c_new[:, 128 * b:128 * b + 128],
                                        ident[:])
                for b in range(4):
                    nc.tensor.transpose(trh[:, 128 * b:128 * b + 128],
                                        h_new[:, 128 * b:128 * b + 128],
                                        ident[:])
                myt = myTp.tile([128, 1024], BF16, tag="myT", name="myT")
                nc.vector.tensor_copy(myt[:, 0:512], trc[:])
                nc.scalar.copy(myt[:, 512:1024], trh[:])
                cin = dramp.tile([128, 1024], BF16, tag="ci", name="ci")
                cout = dramp.tile([2, 128, 1024], BF16, tag="co", name="co")
                nc.sync.dma_start(cin[:], myt[:])
                nc.gpsimd.collective_compute(
                    "AllGather", mybir.AluOpType.bypass,
                    replica_groups=REPLICA_GROUPS,
                    ins=[cin[:].opt()], outs=[cout[:].opt()],
                )
                gath = hTp.tile([128, 2048], BF16, tag=f"g{l}", name=f"g{l}")
                # dst[p, x*1024 + r*512 + n] = cout[r, p, x*512 + n]
                nc.sync.dma_start(
                    gath[:].rearrange("p (x r n) -> p x r n", x=2, r=2),
                    cout.rearrange("r p (x n) -> p x r n", x=2),
                )
                return gath[:, 0:1024], gath[:, 1024:2048]

            # ------------------------------------------------------------------
            x2T = None   # full h_new(L1)^T for W2 rows 0:1024
            for t in range(T):
                xv = xvp.tile([128, 1024], BF16, tag="xv", name="xv")
                nc.sync.dma_start(xv[:], vtp[t])

                # ---- Layer 1 (chunk-major: one stationary, 4 gate MMs) ----
                ps = {g: gpsp.tile([128, 512], F32, tag="g", name="g")
                      for g in GATE_ORDER}
                for k in range(8):         # v chunks
                    for g in GATE_ORDER:
                        nc.tensor.matmul(ps[g][:], xv[:, 128 * k:128 * k + 128],
                                         wt[(1, g)][:, 512 * k:512 * k + 512],
                                         start=(k == 0), stop=False,
                                         skip_group_check=True)
                for k in range(8):         # h-slot chunks (c1_new prev)
                    for g in GATE_ORDER:
                        nc.tensor.matmul(ps[g][:],
                                         hslotT[1][:, 128 * k:128 * k + 128],
                                         wt[(1, g)][:, 512 * (8 + k):512 * (8 + k) + 512],
                                         start=False, stop=False,
                                         skip_group_check=True)
                for g in GATE_ORDER:
                    nc.tensor.matmul(ps[g][:], ones[:],
                                     bat[1][:, 512 * g:512 * g + 512],
                                     start=False, stop=False,
                                     skip_group_check=True)
                for g in GATE_ORDER:
                    nc.tensor.matmul(ps[g][:], out_t[:],
                                     w1ot[:, 512 * g:512 * g + 512],
                                     start=False, stop=True,
                                     skip_group_check=True)
                ps1 = ps

                # ---- Layer 2 independent part (h-slot chunks + bias) ----
                # Emitted before L1's elementwise-dependent transposes so the
                # in-order PE queue runs these while other engines do L1's
                # cell math.
                ps = {g: gpsp.tile([128, 512], F32, tag="g", name="g")
                      for g in GATE_ORDER}
                for k in range(8):         # h-slot chunks (c2_new prev) FIRST
                    for g in GATE_ORDER:
                        nc.tensor.matmul(ps[g][:],
                                         hslotT[2][:, 128 * k:128 * k + 128],
                                         wt[(2, g)][:, 512 * k:512 * k + 512],
                                         start=(k == 0), stop=False,
                                         skip_group_check=True)
                for g in GATE_ORDER:
                    nc.tensor.matmul(ps[g][:], ones[:],
                                     bat[2][:, 512 * g:512 * g + 512],
                                     start=False, stop=False,
                                     skip_group_check=True)

                c1n, h1n = cell_elementwise(1, ps1, c_sl[1])
                c_sl[1] = h1n
                c1T, x2T = exchange(1, c1n, h1n)
                hslotT[1] = c1T

                # ---- Layer 2 dependent part (h1/x chunks) ----
                for k in range(8):         # h1 (x) chunks, wait on L1 exchange
                    for g in GATE_ORDER:
                        nc.tensor.matmul(ps[g][:], x2T[:, 128 * k:128 * k + 128],
                                         wt[(2, g)][:, 512 * (8 + k):512 * (8 + k) + 512],
                                         start=False, stop=(k == 7),
                                         skip_group_check=True)
                c2n, h2n = cell_elementwise(2, ps, c_sl[2])
                c_sl[2] = h2n
                c2T, h2T = exchange(2, c2n, h2n)
                hslotT[2] = c2T

                # ---- output projection: out^T = tanh(Wl^T @ h2 + bl) ----
                pso = opsp.tile([64, 128], F32, tag="op", name="op")
                for k in range(8):
                    nc.tensor.matmul(pso[:], wlt[:, 64 * k:64 * k + 64],
                                     h2T[:, 128 * k:128 * k + 128],
                                     start=(k == 0), stop=False)
                nc.tensor.matmul(pso[:], blt[:], ones[:],
                                 start=False, stop=True)
                ys_sb = outp.tile([64, 128], F32, tag="ysb", name="ysb")
                nc.scalar.activation(ys_sb[:], pso[:], Tanh)
                nc.sync.dma_start(ysT[t], ys_sb[:])
                if t + 1 < T:
                    out_t = outp.tile([64, 128], BF16, tag="outT", name="outT")
                    nc.vector.tensor_copy(out_t[:], ys_sb[:])

    nc.compile()
    return nc


# ---------------------------------------------------------------------------
# host-side input prep
# ---------------------------------------------------------------------------

def _bf16(x):
    return np.asarray(x, dtype=np.float32).astype(ml_dtypes.bfloat16)


def _prep_inputs(inputs, T):
    V = np.asarray(inputs["V_seq"], dtype=np.float32)
    B0 = np.asarray(inputs["B0"], dtype=np.float32)
    init = np.asarray(inputs["initial_state"], dtype=np.float32)
    init = init.reshape(BS, 2, 2, UNITS)
    W1 = np.asarray(inputs["W1"], dtype=np.float32)
    W2 = np.asarray(inputs["W2"], dtype=np.float32)
    b1 = np.asarray(inputs["b1"], dtype=np.float32)
    b2 = np.asarray(inputs["b2"], dtype=np.float32)
    Wl = np.asarray(inputs["Wl"], dtype=np.float32)
    bl = np.asarray(inputs["bl"], dtype=np.float32)

    W1z = W1.copy()
    W1z[VDIM:1024] = 0.0
    arr1 = W1z.reshape(16, 128, 4096)
    arr2 = W2.reshape(16, 128, 4096)

    fb = np.concatenate([np.zeros(2048, np.float32),
                         np.full(1024, FORGET_BIAS, np.float32),
                         np.zeros(1024, np.float32)])
    b1f = b1 + fb
    b2f = b2 + fb

    # Wl^T tile: [p, 64k + o] = Wl[128k + p, o]
    wlT = np.zeros((128, 8, 64), np.float32)
    wlT[:, :, :BDIM] = Wl.reshape(8, 128, BDIM).transpose(1, 0, 2)
    wlT = _bf16(wlT.reshape(128, 512))
    blT = np.zeros((1, 64), np.float32)
    blT[0, :BDIM] = bl
    blT = _bf16(blT)

    in_maps = []
    for c in range(N_CORES):
        p, u = c // 2, c % 2
        b0 = p * BG
        cols = np.concatenate([np.arange(g * 1024 + u * UH, g * 1024 + u * UH + UH)
                               for g in range(GATES)])  # (2048,)

        Vp = np.zeros((T, BG, 1024), np.float32)
        Vp[:, :, :VDIM] = V[:T, b0:b0 + BG, :]
        vtp = Vp.reshape(T, BG, 8, 128).transpose(0, 3, 2, 1).reshape(T, 128, 1024)

        w1c = arr1[:, :, cols].reshape(16, 128, 4, 512)    # [k, p, g, n]
        w1rc = w1c.transpose(2, 1, 0, 3).reshape(4, 128, 8192)
        # L2: chunk order [h-slot rows 1024:2048 (k 8..15) | h1 rows 0:1024]
        w2c = arr2[:, :, cols].reshape(16, 128, 4, 512)
        w2rc = np.concatenate([w2c[8:16], w2c[0:8]], axis=0)
        w2rc = w2rc.transpose(2, 1, 0, 3).reshape(4, 128, 8192)

        w1oc = np.zeros((64, 4, 512), np.float32)
        w1oc[:BDIM] = W1[VDIM:VDIM + BDIM][:, cols].reshape(BDIM, 4, 512)
        w1oc = w1oc.reshape(64, 2048)

        h0T = {}
        for l in (1, 2):
            hs = init[b0:b0 + BG, l - 1, 1, :]          # (128, 1024)
            h0T[l] = hs.T.reshape(8, 128, BG).transpose(1, 0, 2).reshape(128, 1024)

        o0T = np.zeros((64, 128), np.float32)
        o0T[:BDIM] = B0[b0:b0 + BG].T

        m = {
            "vtp": _bf16(vtp),
            "w1r": _bf16(w1rc), "w1o": _bf16(w1oc), "b1a": _bf16(b1f[cols][None, :]),
            "w2r": _bf16(w2rc), "b2a": _bf16(b2f[cols][None, :]),
            "wlT": wlT, "blT": blT,
            "c10": np.ascontiguousarray(init[b0:b0 + BG, 0, 0, u * UH:u * UH + UH]),
            "c20": np.ascontiguousarray(init[b0:b0 + BG, 1, 0, u * UH:u * UH + UH]),
            "h10T": _bf16(h0T[1]), "h20T": _bf16(h0T[2]),
            "o0T": _bf16(o0T),
        }
        for l in (1, 2):
            for nm in ("pi", "pf", "po"):
                v = np.asarray(inputs[f"{nm}{l}"], dtype=np.float32)
                m[f"{nm}{l}"] = _bf16(np.broadcast_to(v[u * UH:u * UH + UH],
                                                      (128, UH)))
        in_maps.append(m)
    return in_maps


# ---------------------------------------------------------------------------
# cached PJRT runner
# ---------------------------------------------------------------------------

_RUNNERS = {}


class _Runner:
    def __init__(self, T):
        import jax
        from jax.sharding import Mesh, PartitionSpec, NamedSharding
        from jax.experimental.shard_map import shard_map
        from concourse import bass2jax

        self.T = T
        nc = _build(T)
        bass2jax.install_neuronx_cc_hook()

        partition_name = (nc.partition_id_tensor.name
                          if nc.partition_id_tensor else None)
        in_names, out_names, out_avals, zero_outs = [], [], [], []
        for alloc in nc.m.functions[0].allocations:
            if not isinstance(alloc, mybir.MemoryLocationSet):
                continue
            name = alloc.memorylocations[0].name
            if alloc.kind == "ExternalInput":
                if name != partition_name:
                    in_names.append(name)
            elif alloc.kind == "ExternalOutput":
                shape = tuple(alloc.tensor_shape)
                dtype = mybir.dt.np(alloc.dtype)
                out_names.append(name)
                out_avals.append(jax.core.ShapedArray(shape, dtype))
                zero_outs.append(np.zeros(shape, dtype))
        self.in_names = list(in_names)
        self.out_names = out_names
        self.zero_outs = zero_outs
        n_params = len(in_names)
        n_outs = len(out_avals)
        all_in_names = list(in_names) + list(out_names)
        if partition_name is not None:
            all_in_names.append(partition_name)

        def _body(*args):
            operands = list(args)
            if partition_name is not None:
                operands.append(bass2jax.partition_id_tensor())
            return tuple(bass2jax._bass_exec_p.bind(
                *operands,
                out_avals=tuple(out_avals),
                in_names=tuple(all_in_names),
                out_names=tuple(out_names),
                lowering_input_output_aliases=(),
                sim_require_finite=True,
                sim_require_nnan=True,
                nc=nc,
            ))

        devices = jax.devices()[:N_CORES]
        assert len(devices) == N_CORES
        self.mesh = Mesh(np.asarray(devices), ("core",))
        in_specs = (PartitionSpec("core"),) * (n_params + n_outs)
        out_specs = (PartitionSpec("core"),) * n_outs
        self.sharding = NamedSharding(self.mesh, PartitionSpec("core"))
        # No donation: output operands are passed as plain inputs (initial
        # values) and reused across calls. The NEFF fully overwrites every
        # output element, so reuse is safe, and it avoids re-staging fresh
        # device buffers through the axon relay on every execution.
        self.fn = jax.jit(
            shard_map(_body, mesh=self.mesh, in_specs=in_specs,
                      out_specs=out_specs, check_rep=False),
            keep_unused=True)
        self.jax = jax
        self._out_bufs = None

    def device_inputs(self, in_maps):
        cat = [np.concatenate([np.asarray(m[n]) for m in in_maps], axis=0)
               for n in self.in_names]
        return [self.jax.device_put(a, self.sharding) for a in cat]

    def zero_out_bufs(self):
        if self._out_bufs is None:
            cat = [np.concatenate([z] * N_CORES, axis=0)
                   for z in self.zero_outs]
            self._out_bufs = [self.jax.device_put(a, self.sharding)
                              for a in cat]
        return self._out_bufs

    def run(self, dev_in, out_bufs):
        outs = self.fn(*dev_in, *out_bufs)
        return [np.asarray(o) for o in outs]


def _get_runner(T):
    if T not in _RUNNERS:
        _RUNNERS[T] = _Runner(T)
    return _RUNNERS[T]


# ---------------------------------------------------------------------------
# public entry point
# ---------------------------------------------------------------------------

def kernel(**inputs) -> np.ndarray:
    T = int(inputs["length"]) - 1
    if T <= 0:
        return np.zeros((max(T, 0), BS, BDIM), dtype=np.float32)
    runner = _get_runner(T)
    in_maps = _prep_inputs(inputs, T)
    dev_in = runner.device_inputs(in_maps)
    outs = runner.run(dev_in, runner.zero_out_bufs())
    ysT_cat = outs[runner.out_names.index("ysT")]   # (8*T, 64, 128)
    per_core = ysT_cat.reshape(N_CORES, T, 64, 128)
    ys = np.empty((T, BS, BDIM), dtype=np.float32)
    for p in range(4):
        ys[:, p * BG:(p + 1) * BG, :] = (
            per_core[2 * p, :, :BDIM, :].transpose(0, 2, 1))
    return ys

